# revision 2
# baseline (speedup 1.0000x reference)
"""Segment mean-pooling (scatter_mean) on 8 Trainium2 NeuronCores.

Strategy (data-parallel rows, per the sharding hint):
  - Host shards the 4M rows of x/index across the 8 cores (500K rows
    each), padding each shard to 62*8192 rows (pad rows route to a
    dump bucket that is never read back).
  - Kernel 1 (bucketize), per core: for each 128-row tile, compute
    each row's bucket (idx >> 9; 196 buckets x 512 segments) and its
    exact rank within the bucket via a strict-lower-triangular matmul
    prefix-count on the tensor engine plus a running per-bucket base
    vector; assemble 34-wide rows [x(32) | 1.0 | idx] and scatter each
    tile with a [128,1]-index indirect DMA into bucket-contiguous
    staging (slots are unique by construction - race-free).
  - Kernel 2 (accumulate), per core: for each bucket, bulk-load its
    staged rows, build a 512-wide one-hot from the stored idx on the
    vector engine, and matmul-accumulate [33, 512] PSUM tiles (32
    feature sums + count), writing a transposed partial table
    [33, 196*512].
  - Host all-reduces the 8 partial tables (sum), divides sums by
    max(count, 1), and transposes to the final [100000, 32] output.
"""
import numpy as np
import concourse.bass as bass
import concourse.bacc as bacc
import concourse.tile as tile
import concourse.mybir as mybir
from concourse.bass_utils import run_bass_kernel_spmd

F32 = mybir.dt.float32
I32 = mybir.dt.int32
OP = mybir.AluOpType

N_ROWS = 4000000
D = 32
NUM_SEGMENTS = 100000
N_CORES = 8
N_CHUNKS = 62          # per-core row chunks of 8192 (62*8192 = 507904)
E = 34                 # staged row: x(32) | 1.0 | idx
SEG_PER_B = 512        # segments per bucket (bucket = idx >> 9)
NB = 196               # normal buckets (196*512 = 100352 >= 100000)
CAP = 3072             # slots per bucket (mean 2560 + ~10 sigma)
DUMP_CAP = 8192        # slots for the padding dump bucket
BW = 200               # bucket one-hot width (padded)
AX_X = mybir.AxisListType.X

_cache = {}


def _k1_build():
    slots_total = NB * CAP + DUMP_CAP
    R = N_CHUNKS * 8192
    nc = bacc.Bacc("TRN2", target_bir_lowering=False, debug=False,
                   num_devices=N_CORES)
    x_d = nc.dram_tensor("x", [R, D], F32, kind="ExternalInput")
    i_d = nc.dram_tensor("idx", [R], I32, kind="ExternalInput")
    tri_d = nc.dram_tensor("tri", [128, 128], F32, kind="ExternalInput")
    ones_d = nc.dram_tensor("ones1", [1, 128], F32, kind="ExternalInput")
    onesc_d = nc.dram_tensor("onescol", [128, 1], F32, kind="ExternalInput")
    iota_d = nc.dram_tensor("iotab", [128, BW], F32, kind="ExternalInput")
    start_d = nc.dram_tensor("startv", [1, BW], F32, kind="ExternalInput")
    stage_d = nc.dram_tensor("staging", [slots_total, E], F32,
                             kind="ExternalOutput")
    with tile.TileContext(nc) as tc:
        with tc.tile_pool(name="const", bufs=1) as cp, \
             tc.tile_pool(name="sbuf", bufs=3) as pool, \
             tc.tile_pool(name="psum", bufs=4, space="PSUM") as pp:
            tri = cp.tile([128, 128], F32)
            nc.sync.dma_start(out=tri[:], in_=tri_d.ap())
            ones1 = cp.tile([1, 128], F32)
            nc.sync.dma_start(out=ones1[:], in_=ones_d.ap())
            onescol = cp.tile([128, 1], F32)
            nc.sync.dma_start(out=onescol[:], in_=onesc_d.ap())
            iota = cp.tile([128, BW], F32)
            nc.sync.dma_start(out=iota[:], in_=iota_d.ap())
            base = cp.tile([1, BW], F32)
            nc.sync.dma_start(out=base[:], in_=start_d.ap())
            for c in range(N_CHUNKS):
                r0 = c * 8192
                xt = pool.tile([128, 64 * D], F32, tag="x")
                nc.sync.dma_start(
                    out=xt[:],
                    in_=x_d.ap()[r0:r0 + 8192, :].rearrange(
                        "(p k) d -> p (k d)", p=128))
                iti = pool.tile([128, 64], I32, tag="ii")
                nc.sync.dma_start(
                    out=iti[:],
                    in_=i_d.ap()[r0:r0 + 8192].rearrange("(p k) -> p k", p=128))
                bbi = pool.tile([128, 64], I32, tag="bi")
                nc.vector.tensor_scalar(out=bbi[:], in0=iti[:], scalar1=9,
                                        scalar2=None, op0=OP.arith_shift_right)
                bbf = pool.tile([128, 64], F32, tag="bf")
                nc.vector.tensor_copy(out=bbf[:], in_=bbi[:])
                idxf = pool.tile([128, 64], F32, tag="if")
                nc.vector.tensor_copy(out=idxf[:], in_=iti[:])
                at = pool.tile([128, 64 * E], F32, tag="at")
                at3 = at[:].rearrange("p (k e) -> p k e", e=E)
                nc.vector.memset(at3[:, :, D:D + 1], 1.0)
                nc.vector.tensor_copy(
                    out=at3[:, :, 0:D],
                    in_=xt[:].rearrange("p (k d) -> p k d", d=D))
                nc.vector.tensor_copy(out=at3[:, :, D + 1:E],
                                      in_=idxf[:].unsqueeze(-1))
                slots_f = pool.tile([128, 64], F32, tag="sf")
                slots_i = pool.tile([128, 64], I32, tag="si")
                for t in range(64):
                    M = pool.tile([128, BW], F32, tag="M")
                    nc.vector.tensor_tensor(
                        out=M[:],
                        in0=bbf[:, t:t + 1].to_broadcast([128, BW]),
                        in1=iota[:], op=OP.is_equal)
                    cum = pp.tile([128, BW], F32, space="PSUM", tag="cum")
                    nc.tensor.matmul(out=cum[:], lhsT=ones1[:], rhs=base[:],
                                     start=True, stop=False)
                    nc.tensor.matmul(out=cum[:], lhsT=tri[:], rhs=M[:],
                                     start=False, stop=True)
                    scr = pool.tile([128, BW], F32, tag="scr")
                    nc.vector.tensor_tensor(out=scr[:], in0=cum[:],
                                            in1=M[:], op=OP.mult)
                    nc.vector.tensor_reduce(out=slots_f[:, t:t + 1],
                                            in_=scr[:], axis=AX_X, op=OP.add)
                    csum = pp.tile([1, BW], F32, space="PSUM", tag="csum")
                    nc.tensor.matmul(out=csum[:], lhsT=onescol[:], rhs=M[:],
                                     start=True, stop=True)
                    nc.vector.tensor_tensor(out=base[:], in0=csum[:],
                                            in1=base[:], op=OP.add)
                    nc.vector.tensor_copy(out=slots_i[:, t:t + 1],
                                          in_=slots_f[:, t:t + 1])
                    nc.gpsimd.indirect_dma_start(
                        out=stage_d.ap(),
                        out_offset=bass.IndirectOffsetOnAxis(
                            ap=slots_i[:, t:t + 1], axis=0),
                        in_=at[:, t * E:(t + 1) * E],
                        in_offset=None)
    nc.compile()
    return nc


def _k2_build():
    slots_total = NB * CAP + DUMP_CAP
    TPB = CAP // 128
    nc = bacc.Bacc("TRN2", target_bir_lowering=False, debug=False,
                   num_devices=N_CORES)
    stage_d = nc.dram_tensor("staging", [slots_total, E], F32,
                             kind="ExternalInput")
    iota_d = nc.dram_tensor("iota512", [128, SEG_PER_B], F32,
                            kind="ExternalInput")
    out_d = nc.dram_tensor("tableT", [D + 1, NB * SEG_PER_B], F32,
                           kind="ExternalOutput")
    with tile.TileContext(nc) as tc:
        with tc.tile_pool(name="const", bufs=1) as cp, \
             tc.tile_pool(name="sbuf", bufs=3) as pool, \
             tc.tile_pool(name="psum", bufs=2, space="PSUM") as pp:
            iota = cp.tile([128, SEG_PER_B], F32)
            nc.sync.dma_start(out=iota[:], in_=iota_d.ap())
            for b in range(NB):
                st = pool.tile([128, TPB * E], F32, tag="st")
                nc.sync.dma_start(
                    out=st[:],
                    in_=stage_d.ap()[b * CAP:(b + 1) * CAP, :].rearrange(
                        "(p r) e -> p (r e)", p=128))
                iob = pool.tile([128, SEG_PER_B], F32, tag="iob")
                nc.vector.tensor_scalar(out=iob[:], in0=iota[:],
                                        scalar1=float(b * SEG_PER_B),
                                        scalar2=None, op0=OP.add)
                ps = pp.tile([D + 1, SEG_PER_B], F32, space="PSUM", tag="ps")
                for t in range(TPB):
                    oh = pool.tile([128, SEG_PER_B], F32, tag="oh")
                    nc.vector.tensor_tensor(
                        out=oh[:],
                        in0=st[:, t * E + D + 1:t * E + E].to_broadcast(
                            [128, SEG_PER_B]),
                        in1=iob[:], op=OP.is_equal)
                    nc.tensor.matmul(out=ps[:], lhsT=st[:, t * E:t * E + D + 1],
                                     rhs=oh[:], start=(t == 0),
                                     stop=(t == TPB - 1))
                ob = pool.tile([D + 1, SEG_PER_B], F32, tag="ob")
                nc.vector.tensor_copy(out=ob[:], in_=ps[:])
                nc.sync.dma_start(
                    out=out_d.ap()[:, b * SEG_PER_B:(b + 1) * SEG_PER_B],
                    in_=ob[:])
    nc.compile()
    return nc


def _consts():
    tri = (np.arange(128)[:, None] < np.arange(128)[None, :]).astype(np.float32)
    ones1 = np.ones((1, 128), np.float32)
    onescol = np.ones((128, 1), np.float32)
    iotab = np.tile(np.arange(BW, dtype=np.float32), (128, 1))
    startv = np.zeros((1, BW), np.float32)
    for b in range(NB):
        startv[0, b] = b * CAP
    for b in range(NB, BW):
        startv[0, b] = NB * CAP  # dump bucket (and unused tail)
    iota512 = np.tile(np.arange(SEG_PER_B, dtype=np.float32), (128, 1))
    return tri, ones1, onescol, iotab, startv, iota512


def kernel(x, index):
    x = np.ascontiguousarray(np.asarray(x, dtype=np.float32))
    idx = np.asarray(index)
    assert x.shape == (N_ROWS, D)
    if "k1" not in _cache:
        _cache["k1"] = _k1_build()
        _cache["k2"] = _k2_build()
    k1, k2 = _cache["k1"], _cache["k2"]
    tri, ones1, onescol, iotab, startv, iota512 = _consts()
    idx32 = idx.astype(np.int32)
    per = N_ROWS // N_CORES
    R = N_CHUNKS * 8192
    for c in range(N_CORES):
        bc = np.bincount(idx32[c * per:(c + 1) * per] >> 9, minlength=NB)
        if bc.max() > CAP:
            raise RuntimeError(
                f"bucket overflow on core {c}: {bc.max()} > {CAP} rows in one "
                f"512-segment bucket (kernel sized for uniform indices)")
    in1 = []
    for c in range(N_CORES):
        xs = np.zeros((R, D), np.float32)
        xs[:per] = x[c * per:(c + 1) * per]
        ii = np.full((R,), NB * SEG_PER_B, np.int32)  # pad -> dump bucket
        ii[:per] = idx32[c * per:(c + 1) * per]
        in1.append({"x": xs, "idx": ii, "tri": tri, "ones1": ones1,
                    "onescol": onescol, "iotab": iotab, "startv": startv})
    r1 = run_bass_kernel_spmd(k1, in1, list(range(N_CORES))).results
    in2 = [{"staging": r1[c]["staging"], "iota512": iota512}
           for c in range(N_CORES)]
    r2 = run_bass_kernel_spmd(k2, in2, list(range(N_CORES))).results
    acc = np.zeros((D + 1, NB * SEG_PER_B), np.float64)
    for c in range(N_CORES):
        acc += r2[c]["tableT"]
    sums = acc[:D, :NUM_SEGMENTS].T
    counts = acc[D, :NUM_SEGMENTS]
    out = sums / np.maximum(counts, 1.0)[:, None]
    return out.astype(np.float32)
